# revision 3
# baseline (speedup 1.0000x reference)
import sys

sys.path.insert(0, "/opt/trn_rl_repo")
import numpy as np

N1, N2, D = 8192, 8192, 256
NCORES = 8
QPC = N1 // NCORES  # queries per core (1024)
RT = QPC // 128  # row tiles per core (8)
GW = 2048  # group width (4 psum banks)
NEG = -1.0e30


def _build_nc():
    import concourse.bass as bass
    import concourse.tile as tile
    from concourse import mybir

    f32, f32r = mybir.dt.float32, mybir.dt.float32r
    nc = bass.Bass()
    dbx = nc.dram_tensor("dbx", [128, 2, 2 * N1], f32r, kind="ExternalInput")
    nrmA = nc.dram_tensor("nrmA", [2, N1 + 128], f32r, kind="ExternalInput")
    nrmB = nc.dram_tensor("nrmB", [2, N1], f32r, kind="ExternalInput")
    dmask = nc.dram_tensor("dmask", [128, 4 * 512], f32, kind="ExternalInput")
    o = nc.dram_tensor("o", [128, RT, 8], f32, kind="ExternalOutput")

    with tile.TileContext(nc) as tc:
        with (
            tc.tile_pool(name="sb", bufs=1) as sb,
            tc.tile_pool(name="pp", bufs=3) as pp,
            tc.tile_pool(name="ps", bufs=2, space="PSUM") as ps,
        ):
            tdb = sb.tile([128, 2, 2 * N1], f32r, tag="db")
            tnrA = sb.tile([2, N1 + 128], f32r, tag="nrA")
            tnrB = sb.tile([2, N1], f32r, tag="nrB")
            tmk = sb.tile([128, 4 * 512], f32, tag="mk")
            nc.sync.dma_start(out=tdb, in_=dbx[:])
            nc.sync.dma_start(out=tnrA, in_=nrmA[:])
            nc.sync.dma_start(out=tnrB, in_=nrmB[:])
            nc.sync.dma_start(out=tmk, in_=dmask[:])
            ones2 = tnrA[0:2, N1 : N1 + 128]
            # wait absorber: DVE observes the dmask DMA once, up front
            dum = sb.tile([128, 1], f32, tag="dum")
            nc.vector.tensor_copy(out=dum, in_=tmk[:, 0:1])
            for m in range(RT):
                lhs = [tdb[:, k, m * 128 : (m + 1) * 128] for k in (0, 1)]
                part = pp.tile([128, 8], f32, tag="part")
                for side in range(2):
                    for g in range(4):
                        col = g * GW
                        pst = ps.tile([128, GW], f32, tag="pst")
                        for k in (0, 1):
                            for i in range(4):
                                nc.tensor.matmul(
                                    out=pst[:, i * 512 : (i + 1) * 512],
                                    lhsT=lhs[k],
                                    rhs=tdb[
                                        :,
                                        k,
                                        side * N1
                                        + col
                                        + i * 512 : side * N1
                                        + col
                                        + (i + 1) * 512,
                                    ],
                                    start=(k == 0),
                                    stop=False,
                                )
                        for i in range(4):
                            nc.tensor.matmul(
                                out=pst[:, i * 512 : (i + 1) * 512],
                                lhsT=ones2,
                                rhs=(tnrA if side == 0 else tnrB)[
                                    0:2, col + i * 512 : col + (i + 1) * 512
                                ],
                                start=False,
                                stop=True,
                            )
                        if side == 0 and g == 0:
                            i0, v = m // 4, m % 4
                            sl = pst[:, i0 * 512 : (i0 + 1) * 512]
                            nc.vector.tensor_add(
                                out=sl, in0=sl, in1=tmk[:, v * 512 : (v + 1) * 512]
                            )
                        nc.vector.tensor_reduce(
                            out=part[:, side * 4 + g : side * 4 + g + 1],
                            in_=pst,
                            axis=mybir.AxisListType.X,
                            op=mybir.AluOpType.max,
                        )
                nc.sync.dma_start(out=o[:, m, :], in_=part)

    from concourse.bass import _bass_rust

    _bass_rust.move_matmul_waits_to_ldweights(nc.m)
    _bass_rust.generate_event_semaphores(nc)
    return nc


def _tf32_hi(x):
    return (x.astype(np.float32).view(np.uint32) & 0xFFFFE000).view(np.float32)


def _prep_core(s1, s2T, sq2hi, sq2lo, c):
    s1p = np.roll(s1, -c * QPC, axis=0)
    dbx = np.empty((128, 2, 2 * N1), dtype=np.float32)
    s1pT = np.ascontiguousarray(s1p.T)
    for k in (0, 1):
        dbx[:, k, 0:N1] = s1pT[k * 128 : (k + 1) * 128]
        dbx[:, k, N1 : 2 * N1] = s2T[k * 128 : (k + 1) * 128]
    nA = (-0.5 * np.square(s1p.astype(np.float64)).sum(1)).astype(np.float32)
    hiA = _tf32_hi(nA)
    nrmA = np.ones((2, N1 + 128), dtype=np.float32)
    nrmA[0, :N1], nrmA[1, :N1] = hiA, nA - hiA
    return dbx, nrmA


def kernel(s1, s2, k):
    assert int(k) == 1
    from concourse.bass_utils import run_bass_kernel_spmd

    s1 = np.asarray(s1, dtype=np.float32)
    s2 = np.asarray(s2, dtype=np.float32)
    nB = (-0.5 * np.square(s2.astype(np.float64)).sum(1)).astype(np.float32)
    hiB = _tf32_hi(nB)
    nrmB = np.stack([hiB, nB - hiB])
    s2T = np.ascontiguousarray(s2.T)
    dmask = np.zeros((128, 4 * 512), dtype=np.float32)
    for v in range(4):
        for p in range(128):
            dmask[p, v * 512 + v * 128 + p] = NEG

    nc = _build_nc()
    in_maps = []
    for c in range(NCORES):
        dbx, nrmA = _prep_core(s1, s2T, hiB, nB - hiB, c)
        in_maps.append({"dbx": dbx, "nrmA": nrmA, "nrmB": nrmB, "dmask": dmask})
    import os
    res = run_bass_kernel_spmd(
        nc, in_maps, core_ids=list(range(NCORES)),
        trace=os.environ.get("KBENCH_TRACE") == "1",
    )
    kernel.last_results = res

    # host epilogue (float64): rho/nu from per-group maxes, then the estimator
    sq1 = np.square(s1.astype(np.float64)).sum(1)
    total = 0.0
    for c in range(NCORES):
        part = res.results[c]["o"].astype(np.float64)  # [128, RT, 8]
        maxA = part[:, :, 0:4].max(axis=2)  # [128, RT]
        maxB = part[:, :, 4:8].max(axis=2)
        idx = np.arange(RT)[None, :] * 128 + np.arange(128)[:, None]
        orig = (c * QPC + idx) % N1
        sqx = sq1[orig]
        rho_sq = sqx - 2.0 * maxA
        nu_sq = sqx - 2.0 * maxB
        rho_sq = np.maximum(rho_sq, 1e-20)
        nu_sq = np.maximum(nu_sq, 1e-20)
        total += 0.5 * (np.log(nu_sq) - np.log(rho_sq)).sum()
    base = np.log(N2 / (N1 - 1))
    return np.float32(base + (D / N1) * total)


# revision 5
# speedup vs baseline: 1.0380x; 1.0380x over previous
import sys

sys.path.insert(0, "/opt/trn_rl_repo")
import numpy as np

N1, N2, D = 8192, 8192, 256
NCORES = 8
QPC = N1 // NCORES  # queries per core (1024)
RT = QPC // 128  # row tiles per core (8)
GW = 2048  # group width (4 psum banks)
NEG = -1.0e30


def _build_nc():
    import concourse.bass as bass
    import concourse.tile as tile
    from concourse import mybir

    f32, f32r = mybir.dt.float32, mybir.dt.float32r
    nc = bass.Bass()
    dbx = nc.dram_tensor("dbx", [128, 2, 2 * N1], f32r, kind="ExternalInput")
    nrmA = nc.dram_tensor("nrmA", [2, N1 + 128], f32r, kind="ExternalInput")
    nrmB = nc.dram_tensor("nrmB", [2, N1], f32r, kind="ExternalInput")
    dmask = nc.dram_tensor("dmask", [128, 4 * 512], f32, kind="ExternalInput")
    o = nc.dram_tensor("o", [128, RT, 8], f32, kind="ExternalOutput")

    with tile.TileContext(nc) as tc:
        with (
            tc.tile_pool(name="sb", bufs=1) as sb,
            tc.tile_pool(name="pp", bufs=1) as pp,
            tc.tile_pool(name="ps", bufs=2, space="PSUM") as ps,
        ):
            CW = 2 * GW  # chunk width (4096 cols)
            chunks = {}
            for side in range(2):
                for h in range(2):
                    t = sb.tile([128, 2, CW], f32r, name=f"db{side}{h}", tag=f"db{side}{h}")
                    off = side * N1 + h * CW
                    nc.sync.dma_start(out=t, in_=dbx[:, :, off : off + CW])
                    chunks[(side, h)] = t
            tnrA = sb.tile([2, N1 + 128], f32r, tag="nrA")
            tnrB = sb.tile([2, N1], f32r, tag="nrB")
            tmk = sb.tile([128, 4 * 512], f32, tag="mk")
            nc.sync.dma_start(out=tnrA, in_=nrmA[:])
            nc.sync.dma_start(out=tnrB, in_=nrmB[:])
            nc.sync.dma_start(out=tmk, in_=dmask[:])
            ones2 = tnrA[0:2, N1 : N1 + 128]
            # wait absorber: DVE observes the dmask DMA once, up front
            dum = sb.tile([128, 1], f32, tag="dum")
            nc.vector.tensor_copy(out=dum, in_=tmk[:, 0:1])
            parts = [pp.tile([128, 8], f32, name=f"part{m}", tag=f"part{m}") for m in range(RT)]
            tq = chunks[(0, 0)]
            for side in range(2):
                for h in range(2):
                    tch = chunks[(side, h)]
                    for m in range(RT):
                        lhs = [tq[:, k, m * 128 : (m + 1) * 128] for k in (0, 1)]
                        part = parts[m]
                        for g2 in range(2):
                            g = h * 2 + g2
                            col = g * GW
                            lcol = g2 * GW
                            pst = ps.tile([128, GW], f32, tag="pst")
                            for k in (0, 1):
                                for i in range(4):
                                    nc.tensor.matmul(
                                        out=pst[:, i * 512 : (i + 1) * 512],
                                        lhsT=lhs[k],
                                        rhs=tch[
                                            :, k, lcol + i * 512 : lcol + (i + 1) * 512
                                        ],
                                        start=(k == 0),
                                        stop=False,
                                    )
                            for i in range(4):
                                nc.tensor.matmul(
                                    out=pst[:, i * 512 : (i + 1) * 512],
                                    lhsT=ones2,
                                    rhs=(tnrA if side == 0 else tnrB)[
                                        0:2, col + i * 512 : col + (i + 1) * 512
                                    ],
                                    start=False,
                                    stop=True,
                                )
                            if side == 0 and g == 0:
                                i0, v = m // 4, m % 4
                                sl = pst[:, i0 * 512 : (i0 + 1) * 512]
                                nc.vector.tensor_add(
                                    out=sl, in0=sl, in1=tmk[:, v * 512 : (v + 1) * 512]
                                )
                            nc.vector.tensor_reduce(
                                out=part[:, side * 4 + g : side * 4 + g + 1],
                                in_=pst,
                                axis=mybir.AxisListType.X,
                                op=mybir.AluOpType.max,
                            )
            for m in range(RT):
                nc.sync.dma_start(out=o[:, m, :], in_=parts[m])

    from concourse.bass import _bass_rust

    _bass_rust.move_matmul_waits_to_ldweights(nc.m)
    _bass_rust.generate_event_semaphores(nc)
    return nc


def _tf32_hi(x):
    return (x.astype(np.float32).view(np.uint32) & 0xFFFFE000).view(np.float32)


def _prep_core(s1, s2T, sq2hi, sq2lo, c):
    s1p = np.roll(s1, -c * QPC, axis=0)
    dbx = np.empty((128, 2, 2 * N1), dtype=np.float32)
    s1pT = np.ascontiguousarray(s1p.T)
    for k in (0, 1):
        dbx[:, k, 0:N1] = s1pT[k * 128 : (k + 1) * 128]
        dbx[:, k, N1 : 2 * N1] = s2T[k * 128 : (k + 1) * 128]
    nA = (-0.5 * np.square(s1p.astype(np.float64)).sum(1)).astype(np.float32)
    hiA = _tf32_hi(nA)
    nrmA = np.ones((2, N1 + 128), dtype=np.float32)
    nrmA[0, :N1], nrmA[1, :N1] = hiA, nA - hiA
    return dbx, nrmA


def kernel(s1, s2, k):
    assert int(k) == 1
    from concourse.bass_utils import run_bass_kernel_spmd

    s1 = np.asarray(s1, dtype=np.float32)
    s2 = np.asarray(s2, dtype=np.float32)
    nB = (-0.5 * np.square(s2.astype(np.float64)).sum(1)).astype(np.float32)
    hiB = _tf32_hi(nB)
    nrmB = np.stack([hiB, nB - hiB])
    s2T = np.ascontiguousarray(s2.T)
    dmask = np.zeros((128, 4 * 512), dtype=np.float32)
    for v in range(4):
        for p in range(128):
            dmask[p, v * 512 + v * 128 + p] = NEG

    nc = _build_nc()
    in_maps = []
    for c in range(NCORES):
        dbx, nrmA = _prep_core(s1, s2T, hiB, nB - hiB, c)
        in_maps.append({"dbx": dbx, "nrmA": nrmA, "nrmB": nrmB, "dmask": dmask})
    import os
    res = run_bass_kernel_spmd(
        nc, in_maps, core_ids=list(range(NCORES)),
        trace=os.environ.get("KBENCH_TRACE") == "1",
    )
    kernel.last_results = res

    # host epilogue (float64): rho/nu from per-group maxes, then the estimator
    sq1 = np.square(s1.astype(np.float64)).sum(1)
    total = 0.0
    for c in range(NCORES):
        part = res.results[c]["o"].astype(np.float64)  # [128, RT, 8]
        maxA = part[:, :, 0:4].max(axis=2)  # [128, RT]
        maxB = part[:, :, 4:8].max(axis=2)
        idx = np.arange(RT)[None, :] * 128 + np.arange(128)[:, None]
        orig = (c * QPC + idx) % N1
        sqx = sq1[orig]
        rho_sq = sqx - 2.0 * maxA
        nu_sq = sqx - 2.0 * maxB
        rho_sq = np.maximum(rho_sq, 1e-20)
        nu_sq = np.maximum(nu_sq, 1e-20)
        total += 0.5 * (np.log(nu_sq) - np.log(rho_sq)).sum()
    base = np.log(N2 / (N1 - 1))
    return np.float32(base + (D / N1) * total)


# revision 7
# speedup vs baseline: 1.0468x; 1.0085x over previous
import sys

sys.path.insert(0, "/opt/trn_rl_repo")
import numpy as np

N1, N2, D = 8192, 8192, 256
NCORES = 8
QPC = N1 // NCORES  # queries per core (1024)
RT = QPC // 128  # row tiles per core (8)
GW = 2048  # group width (4 psum banks)
NEG = -1.0e30


def _build_nc():
    import concourse.bass as bass
    import concourse.tile as tile
    from concourse import mybir

    f32, f32r = mybir.dt.float32, mybir.dt.float32r
    nc = bass.Bass()
    dbx = nc.dram_tensor("dbx", [128, 2, 2 * N1], f32r, kind="ExternalInput")
    nrmA = nc.dram_tensor("nrmA", [2, N1 + 128], f32r, kind="ExternalInput")
    nrmB = nc.dram_tensor("nrmB", [2, N1], f32r, kind="ExternalInput")
    dmask = nc.dram_tensor("dmask", [128, 4 * 512], f32, kind="ExternalInput")
    o = nc.dram_tensor("o", [128, RT, 8], f32, kind="ExternalOutput")

    with tile.TileContext(nc) as tc:
        with (
            tc.tile_pool(name="sb", bufs=1) as sb,
            tc.tile_pool(name="pp", bufs=1) as pp,
            tc.tile_pool(name="ps", bufs=2, space="PSUM") as ps,
        ):
            CW = 2 * GW  # chunk width (4096 cols)
            chunks = {}
            for side in range(2):
                for h in range(2):
                    t = sb.tile([128, 2, CW], f32r, name=f"db{side}{h}", tag=f"db{side}{h}")
                    off = side * N1 + h * CW
                    eng = [nc.sync, nc.scalar, nc.sync, nc.scalar][side * 2 + h]
                    eng.dma_start(out=t, in_=dbx[:, :, off : off + CW])
                    chunks[(side, h)] = t
            tnrA = sb.tile([2, N1 + 128], f32r, tag="nrA")
            tnrB = sb.tile([2, N1], f32r, tag="nrB")
            tmk = sb.tile([128, 4 * 512], f32, tag="mk")
            nc.sync.dma_start(out=tnrA, in_=nrmA[:])
            nc.sync.dma_start(out=tnrB, in_=nrmB[:])
            nc.sync.dma_start(out=tmk, in_=dmask[:])
            ones2 = tnrA[0:2, N1 : N1 + 128]
            # wait absorber: DVE observes the dmask DMA once, up front
            dum = sb.tile([128, 1], f32, tag="dum")
            nc.vector.tensor_copy(out=dum, in_=tmk[:, 0:1])
            parts = [pp.tile([128, 8], f32, name=f"part{m}", tag=f"part{m}") for m in range(RT)]
            tq = chunks[(0, 0)]
            for side in range(2):
                for h in range(2):
                    tch = chunks[(side, h)]
                    for m in range(RT):
                        lhs = [tq[:, k, m * 128 : (m + 1) * 128] for k in (0, 1)]
                        part = parts[m]
                        for g2 in range(2):
                            g = h * 2 + g2
                            col = g * GW
                            lcol = g2 * GW
                            pst = ps.tile([128, GW], f32, tag="pst")
                            for k in (0, 1):
                                for i in range(4):
                                    nc.tensor.matmul(
                                        out=pst[:, i * 512 : (i + 1) * 512],
                                        lhsT=lhs[k],
                                        rhs=tch[
                                            :, k, lcol + i * 512 : lcol + (i + 1) * 512
                                        ],
                                        start=(k == 0),
                                        stop=False,
                                    )
                            for i in range(4):
                                nc.tensor.matmul(
                                    out=pst[:, i * 512 : (i + 1) * 512],
                                    lhsT=ones2,
                                    rhs=(tnrA if side == 0 else tnrB)[
                                        0:2, col + i * 512 : col + (i + 1) * 512
                                    ],
                                    start=False,
                                    stop=True,
                                )
                            if side == 0 and g == 0:
                                i0, v = m // 4, m % 4
                                sl = pst[:, i0 * 512 : (i0 + 1) * 512]
                                nc.vector.tensor_add(
                                    out=sl, in0=sl, in1=tmk[:, v * 512 : (v + 1) * 512]
                                )
                            nc.vector.tensor_reduce(
                                out=part[:, side * 4 + g : side * 4 + g + 1],
                                in_=pst,
                                axis=mybir.AxisListType.X,
                                op=mybir.AluOpType.max,
                            )
            for m in range(RT):
                nc.sync.dma_start(out=o[:, m, :], in_=parts[m])

    from concourse.bass import _bass_rust

    _bass_rust.move_matmul_waits_to_ldweights(nc.m)
    _bass_rust.generate_event_semaphores(nc)
    return nc


def _tf32_hi(x):
    return (x.astype(np.float32).view(np.uint32) & 0xFFFFE000).view(np.float32)


def _prep_core(s1, s2T, sq2hi, sq2lo, c):
    s1p = np.roll(s1, -c * QPC, axis=0)
    dbx = np.empty((128, 2, 2 * N1), dtype=np.float32)
    s1pT = np.ascontiguousarray(s1p.T)
    for k in (0, 1):
        dbx[:, k, 0:N1] = s1pT[k * 128 : (k + 1) * 128]
        dbx[:, k, N1 : 2 * N1] = s2T[k * 128 : (k + 1) * 128]
    nA = (-0.5 * np.square(s1p.astype(np.float64)).sum(1)).astype(np.float32)
    hiA = _tf32_hi(nA)
    nrmA = np.ones((2, N1 + 128), dtype=np.float32)
    nrmA[0, :N1], nrmA[1, :N1] = hiA, nA - hiA
    return dbx, nrmA


def kernel(s1, s2, k):
    assert int(k) == 1
    from concourse.bass_utils import run_bass_kernel_spmd

    s1 = np.asarray(s1, dtype=np.float32)
    s2 = np.asarray(s2, dtype=np.float32)
    nB = (-0.5 * np.square(s2.astype(np.float64)).sum(1)).astype(np.float32)
    hiB = _tf32_hi(nB)
    nrmB = np.stack([hiB, nB - hiB])
    s2T = np.ascontiguousarray(s2.T)
    dmask = np.zeros((128, 4 * 512), dtype=np.float32)
    for v in range(4):
        for p in range(128):
            dmask[p, v * 512 + v * 128 + p] = NEG

    nc = _build_nc()
    in_maps = []
    for c in range(NCORES):
        dbx, nrmA = _prep_core(s1, s2T, hiB, nB - hiB, c)
        in_maps.append({"dbx": dbx, "nrmA": nrmA, "nrmB": nrmB, "dmask": dmask})
    import os
    res = run_bass_kernel_spmd(
        nc, in_maps, core_ids=list(range(NCORES)),
        trace=os.environ.get("KBENCH_TRACE") == "1",
    )
    kernel.last_results = res

    # host epilogue (float64): rho/nu from per-group maxes, then the estimator
    sq1 = np.square(s1.astype(np.float64)).sum(1)
    total = 0.0
    for c in range(NCORES):
        part = res.results[c]["o"].astype(np.float64)  # [128, RT, 8]
        maxA = part[:, :, 0:4].max(axis=2)  # [128, RT]
        maxB = part[:, :, 4:8].max(axis=2)
        idx = np.arange(RT)[None, :] * 128 + np.arange(128)[:, None]
        orig = (c * QPC + idx) % N1
        sqx = sq1[orig]
        rho_sq = sqx - 2.0 * maxA
        nu_sq = sqx - 2.0 * maxB
        rho_sq = np.maximum(rho_sq, 1e-20)
        nu_sq = np.maximum(nu_sq, 1e-20)
        total += 0.5 * (np.log(nu_sq) - np.log(rho_sq)).sum()
    base = np.log(N2 / (N1 - 1))
    return np.float32(base + (D / N1) * total)
